# revision 7
# baseline (speedup 1.0000x reference)
"""Trainium2 Bass kernel for nn_KCLWONegLoss.

Reference math (all f32):
    sums    = embs.sum(axis=1)                          # [64, 512]
    pos[p]  = cos(sums[p], sums[p+8])                   # p in 0..55
    a       = g1[neg1]; b = g2[neg2]                    # [56, 32, 512]
    sim[p,d]= cos over K axis (32) of a[p,:,d], b[p,:,d]
    num     = exp(pos/0.1)
    den     = num + sum_d exp(sim/0.1)
    loss    = 2 * sum_p (log(den) - pos/0.1)

Sharding: data-parallel over the D=64 group axis (8 groups/core) for the
embs reduction; the 56 positive pairs are sharded 7/core, with each core
receiving only its 7*32 gathered rows of g1/g2 (row-gather done host-side
at shard-build time; the device still reads every gathered byte from HBM).
Per-core device outputs: one [8, 513] tile = 8 group-sum vectors (cols
0..511) plus the 8 partial negative-denominator sums (col 512). The final
56 cosines + log-sum (~0.1 Mflop) are assembled on host in float64.

Device-side structure (v2): the embs shard is laid out so partition p
holds rows 16p..16p+15 (32 KB contiguous per partition). Four 1 MB DMA
chunks stream in, alternating between the two HWDGE rings (sync + scalar
sequencers) so issue overhead overlaps. The whole 256-row group reduction
runs on the Tensor engine as 16 accumulating f32r matmuls against a fixed
group-selector (f32r matmul is full-rate at free dim 512), leaving the
Vector engine with only the 6 negative-path elementwise products. No
tensor_reduce anywhere — the baseline's 8x1.1us DVE tail is gone.
"""

import numpy as np

D, NG, DIM = 64, 256, 512
L, K = 8, 32
P = D - L               # 56 positive pairs
TEMP = 0.1
EPS = 1e-8
N_CORES = 8
GPC = D // N_CORES      # 8 groups per core
PPC = P // N_CORES      # 7 pairs per core
ROWS = PPC * K          # 224 gathered rows per core, padded to 256
NROW = GPC * NG         # 2048 embs rows per core
RPP = NROW // 128       # 16 rows per partition
NCH = 4                 # embs DMA chunks (4 r-slices each)

_PROGRAM = None         # cached compiled Bass program
LAST_RESULTS = None     # BassKernelResults of the most recent run (for test.py)


def _build_program():
    import concourse.bass as bass
    import concourse.tile as tile
    from concourse import bacc, mybir

    f32 = mybir.dt.float32
    f32r = mybir.dt.float32r
    AF = mybir.ActivationFunctionType
    nc = bacc.Bacc("TRN2", target_bir_lowering=False, debug=False)

    embs_t = nc.dram_tensor("embs_s", [NROW, DIM], f32, kind="ExternalInput")
    gab_t = nc.dram_tensor("gab", [128, 4, DIM], f32, kind="ExternalInput")
    consts_t = nc.dram_tensor("consts", [128, 24], f32, kind="ExternalInput")
    out_t = nc.dram_tensor("out", [GPC, DIM + 1], f32, kind="ExternalOutput")

    with tile.TileContext(nc) as tc:
        with (
            tc.tile_pool(name="pool", bufs=1) as pool,
            tc.tile_pool(name="psum", bufs=1, space=bass.MemorySpace.PSUM) as psum,
        ):
            # consts columns (see kernel() for values):
            #   0..8   : group selector S[p, g] = 1 iff p//16 == g
            #   8..16  : block-ones for pairs 0..3 (col 8+m = rows 32m..32m+32)
            #   16..24 : block-ones for pairs 4..7 (col 16+4+m likewise)
            consts = pool.tile([128, 24], f32r, tag="consts")
            nc.scalar.dma_start(consts[:], consts_t.ap().bitcast(f32r))
            sel = consts[:, 0:8]
            blk = [consts[:, 8:16], consts[:, 16:24]]

            # --- gather tiles: gb + consts ride the scalar HWDGE ring, the
            # rest rides sync — the two first transfers stream concurrently
            # so the negative path unblocks early, then the sync ring owns
            # the full bandwidth for the embs chunks.
            # gab dram layout is partition-major: [p, t, :] = g1 row t*128+p,
            # [p, 2+t, :] = g2 row t*128+p, so each DMA is 4 KB contiguous
            # per partition (128 fat descriptors).
            ga = pool.tile([128, 2, DIM], f32, tag="ga")
            gb = pool.tile([128, 2, DIM], f32, tag="gb")
            nc.sync.dma_start(ga[:], gab_t.ap()[:, 0:2, :])
            nc.scalar.dma_start(gb[:], gab_t.ap()[:, 2:4, :])

            # --- embs shard: partition p holds rows 16p..16p+15, streamed in
            # chunks of r-slices (8 KB contiguous per partition per slice).
            # The last chunk is a single slice so the post-stream matmul tail
            # is one warm matmul.
            eap = embs_t.ap().rearrange("(p r) d -> p r d", p=128).bitcast(f32r)
            chunk_r = [6, 6, 3, 1]
            assert sum(chunk_r) == RPP
            etiles = []
            r0 = 0
            for c, nr in enumerate(chunk_r):
                e = pool.tile([128, nr, DIM], f32r, tag=f"e{c}")
                nc.sync.dma_start(e[:], eap[:, r0:r0 + nr, :])
                etiles.append((e, nr))
                r0 += nr

            # --- negative path: prod/asq/bsq elementwise on DVE, K-block
            # reduction on PE ---
            dot_ps = psum.tile([8, DIM], f32, tag="dot")
            asq_ps = psum.tile([8, DIM], f32, tag="asq")
            bsq_ps = psum.tile([8, DIM], f32, tag="bsq")
            for t in range(2):
                a, b = ga[:, t, :], gb[:, t, :]
                prod = pool.tile([128, DIM], f32r, tag=f"prod{t}")
                aa = pool.tile([128, DIM], f32r, tag=f"aa{t}")
                bb = pool.tile([128, DIM], f32r, tag=f"bb{t}")
                nc.vector.tensor_mul(prod[:], a, b)
                nc.vector.tensor_mul(aa[:], a, a)
                nc.vector.tensor_mul(bb[:], b, b)
                st, sp = (t == 0), (t == 1)
                nc.tensor.matmul(dot_ps[:], blk[t], prod[:], start=st, stop=sp)
                nc.tensor.matmul(asq_ps[:], blk[t], aa[:], start=st, stop=sp)
                nc.tensor.matmul(bsq_ps[:], blk[t], bb[:], start=st, stop=sp)

            # --- group sums: 16 accumulating selector-matmuls, no DVE ---
            sums_ps = psum.tile([GPC, DIM], f32, tag="sums")
            r = 0
            for e, nr in etiles:
                for j in range(nr):
                    nc.tensor.matmul(
                        sums_ps[:],
                        sel,
                        e[:, j, :],
                        start=(r == 0),
                        stop=(r == RPP - 1),
                    )
                    r += 1

            # --- epilogue: sim = dot * rsqrt(asq) * rsqrt(bsq).
            # (gather pad rows are 1.0 so asq/bsq are never 0; the reference
            # eps guard can never bind for randn inputs)
            out_sb = pool.tile([GPC, DIM + 1], f32, tag="out_sb")
            ai = pool.tile([8, DIM], f32, tag="ai")
            bi = pool.tile([8, DIM], f32, tag="bi")
            nc.scalar.activation(ai[:], asq_ps[:], AF.Abs_reciprocal_sqrt)
            nc.scalar.activation(bi[:], bsq_ps[:], AF.Abs_reciprocal_sqrt)
            tmp = pool.tile([8, DIM], f32, tag="tmp")
            nc.vector.tensor_mul(tmp[:], dot_ps[:], ai[:])
            sim = pool.tile([8, DIM], f32, tag="sim")
            nc.vector.tensor_mul(sim[:], tmp[:], bi[:])
            # e = exp(sim/TEMP); den = row-sum(e) lands in out column 512.
            # The den column ships out mid-stream on the scalar ring; only
            # the [8, 512] sums DMA remains after the final matmul.
            ex = pool.tile([8, DIM], f32, tag="ex")
            nc.scalar.activation(
                ex[:], sim[:], AF.Exp,
                scale=float(1.0 / TEMP), accum_out=out_sb[:, DIM:DIM + 1],
            )
            nc.scalar.dma_start(out_t.ap()[:, DIM:DIM + 1], out_sb[:, DIM:DIM + 1])
            nc.scalar.copy(out_sb[:, 0:DIM], sums_ps[:])
            nc.sync.dma_start(out_t.ap()[:, 0:DIM], out_sb[:, 0:DIM])

    nc.compile()
    return nc


def _get_program():
    global _PROGRAM
    if _PROGRAM is None:
        _PROGRAM = _build_program()
    return _PROGRAM


def kernel(embs, g0, g1, g2, neg1, neg2, **_unused):
    global LAST_RESULTS
    from concourse.bass_utils import run_bass_kernel_spmd

    embs = np.ascontiguousarray(np.asarray(embs, dtype=np.float32))
    g1 = np.ascontiguousarray(np.asarray(g1, dtype=np.float32))
    g2 = np.ascontiguousarray(np.asarray(g2, dtype=np.float32))
    neg1 = np.asarray(neg1).astype(np.int64)
    neg2 = np.asarray(neg2).astype(np.int64)

    consts = np.zeros((128, 24), np.float32)
    for g in range(GPC):
        consts[16 * g:16 * (g + 1), g] = 1.0    # group selector
    for m in range(4):
        consts[m * 32:(m + 1) * 32, 8 + m] = 1.0        # pairs 0..3 (tile 0)
        consts[m * 32:(m + 1) * 32, 16 + 4 + m] = 1.0   # pairs 4..7 (tile 1)

    in_maps = []
    for c in range(N_CORES):
        # pad rows are 1.0: the fake 8th pair then has asq=bsq=K exactly,
        # keeping rsqrt finite (its den column is discarded host-side)
        ga = np.ones((2, 128, DIM), np.float32)
        gb = np.ones((2, 128, DIM), np.float32)
        idx1 = neg1[c * PPC:(c + 1) * PPC].reshape(-1)
        idx2 = neg2[c * PPC:(c + 1) * PPC].reshape(-1)
        ga.reshape(256, DIM)[:ROWS] = g1[idx1]
        gb.reshape(256, DIM)[:ROWS] = g2[idx2]
        gab = np.ascontiguousarray(
            np.concatenate([ga, gb], axis=0).transpose(1, 0, 2)
        )                                       # [128, 4, 512], partition-major
        emb_c = embs[c * GPC:(c + 1) * GPC].reshape(NROW, DIM)
        in_maps.append({
            "embs_s": emb_c,
            "gab": gab,
            "consts": consts,
        })

    nc = _get_program()
    res = run_bass_kernel_spmd(nc, in_maps, core_ids=list(range(N_CORES)))
    LAST_RESULTS = res

    outs = [res.results[c]["out"] for c in range(N_CORES)]
    sums = np.concatenate(
        [o[:, :DIM] for o in outs], axis=0
    ).astype(np.float64)                                   # [64, 512]
    den_neg = np.concatenate(
        [o[:PPC, DIM] for o in outs]
    ).astype(np.float64)                                   # [56]

    s_i, s_j = sums[:P], sums[L:]
    na = np.maximum(np.sqrt((s_i * s_i).sum(1)), EPS)
    nb = np.maximum(np.sqrt((s_j * s_j).sum(1)), EPS)
    pos = (s_i * s_j).sum(1) / (na * nb)
    num = np.exp(pos / TEMP)
    den = num + den_neg
    total = 2.0 * np.sum(np.log(den) - pos / TEMP)
    return np.asarray(total, dtype=np.float32)


# revision 8
# speedup vs baseline: 1.1670x; 1.1670x over previous
"""Trainium2 Bass kernel for nn_KCLWONegLoss.

Reference math (all f32):
    sums    = embs.sum(axis=1)                          # [64, 512]
    pos[p]  = cos(sums[p], sums[p+8])                   # p in 0..55
    a       = g1[neg1]; b = g2[neg2]                    # [56, 32, 512]
    sim[p,d]= cos over K axis (32) of a[p,:,d], b[p,:,d]
    num     = exp(pos/0.1)
    den     = num + sum_d exp(sim/0.1)
    loss    = 2 * sum_p (log(den) - pos/0.1)

Sharding: data-parallel over the D=64 group axis (8 groups/core) for the
embs reduction; the 56 positive pairs are sharded 7/core, with each core
receiving only its 7*32 gathered rows of g1/g2 (row-gather done host-side
at shard-build time; the device still reads every gathered byte from HBM).
Per-core device output: one [8, 513] tile = 8 group-sum vectors (cols
0..511) plus the 8 partial negative-denominator sums (col 512). The final
56 cosines + log-sum (~0.1 Mflop) are assembled on host in float64.

Device structure (v5): the entire 2048-row embs reduction runs on the
Tensor engine as 16 accumulating f32r matmuls against per-group ones
columns (f32r matmul is full-rate at free dim 512) — the Vector engine
only does the 6 negative-path elementwise products, all hidden under the
DMA stream. All input DMAs ride one HWDGE ring (sync) in dependency
order: splitting across the two rings starves whichever ring is loaded
second, and the SDMA engines saturate regardless of issue rate. The
slice-major embs view keeps 2 KB descriptors (measured cleanest engine
behavior). The final chunk is one 128-row slice so only a single warm
matmul + [8,513] store remain after the stream ends.
"""

import numpy as np

D, NG, DIM = 64, 256, 512
L, K = 8, 32
P = D - L               # 56 positive pairs
TEMP = 0.1
EPS = 1e-8
N_CORES = 8
GPC = D // N_CORES      # 8 groups per core
PPC = P // N_CORES      # 7 pairs per core
ROWS = PPC * K          # 224 gathered rows per core, padded to 256
NROW = GPC * NG         # 2048 embs rows per core
NSLICE = NROW // 128    # 16 slices of 128 rows; slice s = group s//2

_PROGRAM = None         # cached compiled Bass program
LAST_RESULTS = None     # BassKernelResults of the most recent run (for test.py)


def _build_program():
    import concourse.bass as bass
    import concourse.tile as tile
    from concourse import bacc, mybir

    f32 = mybir.dt.float32
    f32r = mybir.dt.float32r
    AF = mybir.ActivationFunctionType
    nc = bacc.Bacc("TRN2", target_bir_lowering=False, debug=False)

    embs_t = nc.dram_tensor("embs_s", [NROW, DIM], f32, kind="ExternalInput")
    gab_t = nc.dram_tensor("gab", [4, 128, DIM], f32, kind="ExternalInput")
    consts_t = nc.dram_tensor("consts", [128, 80], f32, kind="ExternalInput")
    out_t = nc.dram_tensor("out", [GPC, DIM + 1], f32, kind="ExternalOutput")

    with tile.TileContext(nc) as tc:
        with (
            tc.tile_pool(name="pool", bufs=1) as pool,
            tc.tile_pool(name="psum", bufs=1, space=bass.MemorySpace.PSUM) as psum,
        ):
            # consts columns (see kernel() for values):
            #   8g..8g+8 : selector S_g — all-ones in column g, else 0
            #   64..72   : block-ones for pairs 0..3 (col 64+m = rows 32m..32m+32)
            #   72..80   : block-ones for pairs 4..7 (col 72+4+m likewise)
            consts = pool.tile([128, 80], f32r, tag="consts")
            nc.sync.dma_start(consts[:], consts_t.ap().bitcast(f32r))
            blk = [consts[:, 64:72], consts[:, 72:80]]

            # negative-path gather rows, one packed DMA (2 KB descriptors)
            gab = pool.tile([128, 4, DIM], f32, tag="gab")
            nc.sync.dma_start(gab[:], gab_t.ap().rearrange("t p d -> p t d"))
            ab = [(gab[:, 0, :], gab[:, 2, :]), (gab[:, 1, :], gab[:, 3, :])]

            # embs shard, slice-major: [p, s] = row s*128 + p, so slice s is
            # one matmul rhs and group(s) = s//2. Streamed in chunks; the
            # last chunk is a single slice to minimize the post-stream tail.
            eap = embs_t.ap().rearrange("(s p) d -> p s d", p=128).bitcast(f32r)
            chunk_s = [6, 6, 3, 1]
            assert sum(chunk_s) == NSLICE
            etiles = []
            s0 = 0
            for c, ns in enumerate(chunk_s):
                e = pool.tile([128, ns, DIM], f32r, tag=f"e{c}")
                nc.sync.dma_start(e[:], eap[:, s0:s0 + ns, :])
                etiles.append((e, ns))
                s0 += ns

            # --- negative path: prod/asq/bsq elementwise on DVE, K-block
            # reduction on PE ---
            dot_ps = psum.tile([8, DIM], f32, tag="dot")
            asq_ps = psum.tile([8, DIM], f32, tag="asq")
            bsq_ps = psum.tile([8, DIM], f32, tag="bsq")
            for t, (a, b) in enumerate(ab):
                prod = pool.tile([128, DIM], f32r, tag=f"prod{t}")
                aa = pool.tile([128, DIM], f32r, tag=f"aa{t}")
                bb = pool.tile([128, DIM], f32r, tag=f"bb{t}")
                nc.vector.tensor_mul(prod[:], a, b)
                nc.vector.tensor_mul(aa[:], a, a)
                nc.vector.tensor_mul(bb[:], b, b)
                st, sp = (t == 0), (t == 1)
                nc.tensor.matmul(dot_ps[:], blk[t], prod[:], start=st, stop=sp)
                nc.tensor.matmul(asq_ps[:], blk[t], aa[:], start=st, stop=sp)
                nc.tensor.matmul(bsq_ps[:], blk[t], bb[:], start=st, stop=sp)

            # --- group sums: 16 accumulating selector-matmuls, no DVE ---
            sums_ps = psum.tile([GPC, DIM], f32, tag="sums")
            s = 0
            for e, ns in etiles:
                for j in range(ns):
                    g = s // 2
                    nc.tensor.matmul(
                        sums_ps[:],
                        consts[:, 8 * g:8 * g + 8],
                        e[:, j, :],
                        start=(s == 0),
                        stop=(s == NSLICE - 1),
                    )
                    s += 1

            # --- epilogue: sim = dot * rsqrt(asq) * rsqrt(bsq).
            # (gather pad rows are 1.0 so asq/bsq are never 0; the reference
            # eps guard can never bind for randn inputs)
            out_sb = pool.tile([GPC, DIM + 1], f32, tag="out_sb")
            ai = pool.tile([8, DIM], f32, tag="ai")
            bi = pool.tile([8, DIM], f32, tag="bi")
            nc.scalar.activation(ai[:], asq_ps[:], AF.Abs_reciprocal_sqrt)
            nc.scalar.activation(bi[:], bsq_ps[:], AF.Abs_reciprocal_sqrt)
            tmp = pool.tile([8, DIM], f32, tag="tmp")
            nc.vector.tensor_mul(tmp[:], dot_ps[:], ai[:])
            sim = pool.tile([8, DIM], f32, tag="sim")
            nc.vector.tensor_mul(sim[:], tmp[:], bi[:])
            # e = exp(sim/TEMP); den = row-sum(e) lands in out column 512
            ex = pool.tile([8, DIM], f32, tag="ex")
            nc.scalar.activation(
                ex[:], sim[:], AF.Exp,
                scale=float(1.0 / TEMP), accum_out=out_sb[:, DIM:DIM + 1],
            )
            nc.scalar.copy(out_sb[:, 0:DIM], sums_ps[:])
            nc.sync.dma_start(out_t.ap(), out_sb[:])

    nc.compile()
    return nc


def _get_program():
    global _PROGRAM
    if _PROGRAM is None:
        _PROGRAM = _build_program()
    return _PROGRAM


def kernel(embs, g0, g1, g2, neg1, neg2, **_unused):
    global LAST_RESULTS
    from concourse.bass_utils import run_bass_kernel_spmd

    embs = np.ascontiguousarray(np.asarray(embs, dtype=np.float32))
    g1 = np.ascontiguousarray(np.asarray(g1, dtype=np.float32))
    g2 = np.ascontiguousarray(np.asarray(g2, dtype=np.float32))
    neg1 = np.asarray(neg1).astype(np.int64)
    neg2 = np.asarray(neg2).astype(np.int64)

    consts = np.zeros((128, 80), np.float32)
    for g in range(GPC):
        consts[:, 8 * g + g] = 1.0          # selector S_g, column g
    for m in range(4):
        consts[m * 32:(m + 1) * 32, 64 + m] = 1.0
        consts[m * 32:(m + 1) * 32, 72 + 4 + m] = 1.0

    in_maps = []
    for c in range(N_CORES):
        # pad rows are 1.0: the fake 8th pair then has asq=bsq=K exactly,
        # keeping rsqrt finite (its den column is discarded host-side)
        gab = np.ones((4, 128, DIM), np.float32)
        idx1 = neg1[c * PPC:(c + 1) * PPC].reshape(-1)
        idx2 = neg2[c * PPC:(c + 1) * PPC].reshape(-1)
        gab[:2].reshape(256, DIM)[:ROWS] = g1[idx1]
        gab[2:].reshape(256, DIM)[:ROWS] = g2[idx2]
        emb_c = embs[c * GPC:(c + 1) * GPC].reshape(NROW, DIM)
        in_maps.append({
            "embs_s": emb_c,
            "gab": gab,
            "consts": consts,
        })

    nc = _get_program()
    res = run_bass_kernel_spmd(nc, in_maps, core_ids=list(range(N_CORES)))
    LAST_RESULTS = res

    outs = [res.results[c]["out"] for c in range(N_CORES)]
    sums = np.concatenate(
        [o[:, :DIM] for o in outs], axis=0
    ).astype(np.float64)                                   # [64, 512]
    den_neg = np.concatenate(
        [o[:PPC, DIM] for o in outs]
    ).astype(np.float64)                                   # [56]

    s_i, s_j = sums[:P], sums[L:]
    na = np.maximum(np.sqrt((s_i * s_i).sum(1)), EPS)
    nb = np.maximum(np.sqrt((s_j * s_j).sum(1)), EPS)
    pos = (s_i * s_j).sum(1) / (na * nb)
    num = np.exp(pos / TEMP)
    den = num + den_neg
    total = 2.0 * np.sum(np.log(den) - pos / TEMP)
    return np.asarray(total, dtype=np.float32)


# revision 9
# speedup vs baseline: 1.1971x; 1.0258x over previous
"""Trainium2 Bass kernel for nn_KCLWONegLoss.

Reference math (all f32):
    sums    = embs.sum(axis=1)                          # [64, 512]
    pos[p]  = cos(sums[p], sums[p+8])                   # p in 0..55
    a       = g1[neg1]; b = g2[neg2]                    # [56, 32, 512]
    sim[p,d]= cos over K axis (32) of a[p,:,d], b[p,:,d]
    num     = exp(pos/0.1)
    den     = num + sum_d exp(sim/0.1)
    loss    = 2 * sum_p (log(den) - pos/0.1)

Sharding: data-parallel over the D=64 group axis (8 groups/core) for the
embs reduction; the 56 positive pairs are sharded 7/core, with each core
receiving only its 7*32 gathered rows of g1/g2 (row-gather done host-side
at shard-build time; the device still reads every gathered byte from HBM).
Per-core device output: one [8, 513] tile = 8 group-sum vectors (cols
0..511) plus the 8 partial negative-denominator sums (col 512). The final
56 cosines + log-sum (~0.1 Mflop) are assembled on host in float64.

Device structure (v5): the entire 2048-row embs reduction runs on the
Tensor engine as 16 accumulating f32r matmuls against per-group ones
columns (f32r matmul is full-rate at free dim 512) — the Vector engine
only does the 6 negative-path elementwise products, all hidden under the
DMA stream. All input DMAs ride one HWDGE ring (sync) in dependency
order: splitting across the two rings starves whichever ring is loaded
second, and the SDMA engines saturate regardless of issue rate. The
slice-major embs view keeps 2 KB descriptors (measured cleanest engine
behavior). The final chunk is one 128-row slice so only a single warm
matmul + [8,513] store remain after the stream ends.
"""

import numpy as np

D, NG, DIM = 64, 256, 512
L, K = 8, 32
P = D - L               # 56 positive pairs
TEMP = 0.1
EPS = 1e-8
N_CORES = 8
GPC = D // N_CORES      # 8 groups per core
PPC = P // N_CORES      # 7 pairs per core
ROWS = PPC * K          # 224 gathered rows per core, padded to 256
NROW = GPC * NG         # 2048 embs rows per core
NSLICE = NROW // 128    # 16 slices of 128 rows; slice s = group s//2

_PROGRAM = None         # cached compiled Bass program
LAST_RESULTS = None     # BassKernelResults of the most recent run (for test.py)


def _build_program():
    import concourse.bass as bass
    import concourse.tile as tile
    from concourse import bacc, mybir

    f32 = mybir.dt.float32
    f32r = mybir.dt.float32r
    AF = mybir.ActivationFunctionType
    nc = bacc.Bacc("TRN2", target_bir_lowering=False, debug=False)

    embs_t = nc.dram_tensor("embs_s", [NROW, DIM], f32, kind="ExternalInput")
    gab_t = nc.dram_tensor("gab", [4, 128, DIM], f32, kind="ExternalInput")
    consts_t = nc.dram_tensor("consts", [128, 80], f32, kind="ExternalInput")
    out_t = nc.dram_tensor("out", [GPC, DIM + 1], f32, kind="ExternalOutput")

    with tile.TileContext(nc) as tc:
        with (
            tc.tile_pool(name="pool", bufs=1) as pool,
            tc.tile_pool(name="psum", bufs=1, space=bass.MemorySpace.PSUM) as psum,
        ):
            # consts columns (see kernel() for values):
            #   8g..8g+8 : selector S_g — all-ones in column g, else 0
            #   64..72   : block-ones for pairs 0..3 (col 64+m = rows 32m..32m+32)
            #   72..80   : block-ones for pairs 4..7 (col 72+4+m likewise)
            # consts ride the scalar HWDGE ring: it is served second-class
            # when the sync ring has backlog, but 40 KB still lands well
            # before the first consumer matmul, and it frees the sync
            # sequencer + ring for the bulk stream.
            consts = pool.tile([128, 80], f32r, tag="consts")
            nc.scalar.dma_start(consts[:], consts_t.ap().bitcast(f32r))
            blk = [consts[:, 64:72], consts[:, 72:80]]

            # negative-path gather rows, one packed DMA (2 KB descriptors)
            gab = pool.tile([128, 4, DIM], f32, tag="gab")
            nc.sync.dma_start(gab[:], gab_t.ap().rearrange("t p d -> p t d"))
            ab = [(gab[:, 0, :], gab[:, 2, :]), (gab[:, 1, :], gab[:, 3, :])]

            # embs shard, slice-major: [p, s] = row s*128 + p, so slice s is
            # one matmul rhs and group(s) = s//2. Streamed in chunks; the
            # last chunk is a single slice to minimize the post-stream tail.
            eap = embs_t.ap().rearrange("(s p) d -> p s d", p=128).bitcast(f32r)
            chunk_s = [6, 6, 3, 1]
            assert sum(chunk_s) == NSLICE
            etiles = []
            s0 = 0
            for c, ns in enumerate(chunk_s):
                e = pool.tile([128, ns, DIM], f32r, tag=f"e{c}")
                nc.sync.dma_start(e[:], eap[:, s0:s0 + ns, :])
                etiles.append((e, ns))
                s0 += ns

            # --- negative path: prod/asq/bsq elementwise on DVE, K-block
            # reduction on PE ---
            dot_ps = psum.tile([8, DIM], f32, tag="dot")
            asq_ps = psum.tile([8, DIM], f32, tag="asq")
            bsq_ps = psum.tile([8, DIM], f32, tag="bsq")
            for t, (a, b) in enumerate(ab):
                prod = pool.tile([128, DIM], f32r, tag=f"prod{t}")
                aa = pool.tile([128, DIM], f32r, tag=f"aa{t}")
                bb = pool.tile([128, DIM], f32r, tag=f"bb{t}")
                nc.vector.tensor_mul(prod[:], a, b)
                nc.vector.tensor_mul(aa[:], a, a)
                nc.vector.tensor_mul(bb[:], b, b)
                st, sp = (t == 0), (t == 1)
                nc.tensor.matmul(dot_ps[:], blk[t], prod[:], start=st, stop=sp)
                nc.tensor.matmul(asq_ps[:], blk[t], aa[:], start=st, stop=sp)
                nc.tensor.matmul(bsq_ps[:], blk[t], bb[:], start=st, stop=sp)

            # --- group sums: 16 accumulating selector-matmuls, no DVE ---
            sums_ps = psum.tile([GPC, DIM], f32, tag="sums")
            s = 0
            for e, ns in etiles:
                for j in range(ns):
                    g = s // 2
                    nc.tensor.matmul(
                        sums_ps[:],
                        consts[:, 8 * g:8 * g + 8],
                        e[:, j, :],
                        start=(s == 0),
                        stop=(s == NSLICE - 1),
                    )
                    s += 1

            # --- epilogue: sim = dot * rsqrt(asq) * rsqrt(bsq).
            # (gather pad rows are 1.0 so asq/bsq are never 0; the reference
            # eps guard can never bind for randn inputs)
            out_sb = pool.tile([GPC, DIM + 1], f32, tag="out_sb")
            ai = pool.tile([8, DIM], f32, tag="ai")
            bi = pool.tile([8, DIM], f32, tag="bi")
            nc.scalar.activation(ai[:], asq_ps[:], AF.Abs_reciprocal_sqrt)
            nc.scalar.activation(bi[:], bsq_ps[:], AF.Abs_reciprocal_sqrt)
            tmp = pool.tile([8, DIM], f32, tag="tmp")
            nc.vector.tensor_mul(tmp[:], dot_ps[:], ai[:])
            sim = pool.tile([8, DIM], f32, tag="sim")
            nc.vector.tensor_mul(sim[:], tmp[:], bi[:])
            # e = exp(sim/TEMP); den = row-sum(e) lands in out column 512
            ex = pool.tile([8, DIM], f32, tag="ex")
            nc.scalar.activation(
                ex[:], sim[:], AF.Exp,
                scale=float(1.0 / TEMP), accum_out=out_sb[:, DIM:DIM + 1],
            )
            nc.scalar.copy(out_sb[:, 0:DIM], sums_ps[:])
            nc.sync.dma_start(out_t.ap(), out_sb[:])

    nc.compile()
    return nc


def _get_program():
    global _PROGRAM
    if _PROGRAM is None:
        _PROGRAM = _build_program()
    return _PROGRAM


def kernel(embs, g0, g1, g2, neg1, neg2, **_unused):
    global LAST_RESULTS
    from concourse.bass_utils import run_bass_kernel_spmd

    embs = np.ascontiguousarray(np.asarray(embs, dtype=np.float32))
    g1 = np.ascontiguousarray(np.asarray(g1, dtype=np.float32))
    g2 = np.ascontiguousarray(np.asarray(g2, dtype=np.float32))
    neg1 = np.asarray(neg1).astype(np.int64)
    neg2 = np.asarray(neg2).astype(np.int64)

    consts = np.zeros((128, 80), np.float32)
    for g in range(GPC):
        consts[:, 8 * g + g] = 1.0          # selector S_g, column g
    for m in range(4):
        consts[m * 32:(m + 1) * 32, 64 + m] = 1.0
        consts[m * 32:(m + 1) * 32, 72 + 4 + m] = 1.0

    in_maps = []
    for c in range(N_CORES):
        # pad rows are 1.0: the fake 8th pair then has asq=bsq=K exactly,
        # keeping rsqrt finite (its den column is discarded host-side)
        gab = np.ones((4, 128, DIM), np.float32)
        idx1 = neg1[c * PPC:(c + 1) * PPC].reshape(-1)
        idx2 = neg2[c * PPC:(c + 1) * PPC].reshape(-1)
        gab[:2].reshape(256, DIM)[:ROWS] = g1[idx1]
        gab[2:].reshape(256, DIM)[:ROWS] = g2[idx2]
        emb_c = embs[c * GPC:(c + 1) * GPC].reshape(NROW, DIM)
        in_maps.append({
            "embs_s": emb_c,
            "gab": gab,
            "consts": consts,
        })

    nc = _get_program()
    res = run_bass_kernel_spmd(nc, in_maps, core_ids=list(range(N_CORES)))
    LAST_RESULTS = res

    outs = [res.results[c]["out"] for c in range(N_CORES)]
    sums = np.concatenate(
        [o[:, :DIM] for o in outs], axis=0
    ).astype(np.float64)                                   # [64, 512]
    den_neg = np.concatenate(
        [o[:PPC, DIM] for o in outs]
    ).astype(np.float64)                                   # [56]

    s_i, s_j = sums[:P], sums[L:]
    na = np.maximum(np.sqrt((s_i * s_i).sum(1)), EPS)
    nb = np.maximum(np.sqrt((s_j * s_j).sum(1)), EPS)
    pos = (s_i * s_j).sum(1) / (na * nb)
    num = np.exp(pos / TEMP)
    den = num + den_neg
    total = 2.0 * np.sum(np.log(den) - pos / TEMP)
    return np.asarray(total, dtype=np.float32)
